# revision 48
# baseline (speedup 1.0000x reference)
"""Multi-head attention (EMBED=384, 6 heads, S=1024, N=16) on 8 trn2 NeuronCores.

Strategy: data-parallel over batch (2 batches/core). Everything stays on-chip
per batch. Layouts chosen so no transposes are ever needed:
  - x[b] is [C, S] in DRAM == tokens^T, used directly as matmul rhs/lhsT;
    x and w_qkv are shipped as bf16 (half the DMA bytes at the same 1
    cycle/row PE rate -- the cost model serializes all DMA transfers on one
    exclusive device, so startup is transfer-bytes-bound).
  - qT/kT computed as [C_qk, S] (w_qk @ x) -> scoresT tiles = kT-tile.T @ qT,
    with the two heads of a pair packed into PE row groups (K=64 each).
  - scores psum is [128, 1024] (two banks); exp runs once per head per t at
    1024 wide (halves the ACT per-instruction overhead) and writes the attn
    weights directly as fp8e4 (e4m3).
  - v is computed in [S, C_v] layout (x-tile.T @ w_vT) and split into fp8 hi
    (vh = e4(v), plus a ones column for the softmax row sums) and fp8 lo
    (vl = e4(v - vh), zeros column) parts.
  - attn@v runs as fp8 DoubleRow matmul chains (two 128-deep k-tiles per
    instruction at 0.5 cycles/row): vh-chain then vl-chain accumulate into one
    [65, 512] psum; row 64 collects the fp8-consistent softmax sums. The v
    hi/lo split cancels the v quantization error, so the only fp8 noise left
    is the attn-weight rounding (~1.5e-2 max rel, inside the 2e-2 budget).
  - exp on ACT with the 1/sqrt(d) scale folded in; no max-subtraction needed
    (|scores*scale| < ~6, exp is safe).
  - normalization: reciprocal of sums -> broadcast across 64 partitions via a
    tiny K=2 selector matmul -> elementwise multiply on catT; the output
    projection then emits finalT = [C, S], DMA'd straight to the output.
  - the per-head softmax reciprocals are written on partition 64 (same
    partition as the psum sums row) and broadcast across catT partitions by
    K=1 selector matmuls at tile_position row 64 -- no cross-partition DMA.
  - the output projection contracts k-tiles 0,1 into SBUF partials (with the
    bias folded in) as soon as their norms land; after the last norm only a
    K=128 matmul plus one DVE add per output tile remains.
Scheduling: a single work FIFO (queue order == emission order, which the
tile framework requires for read-after-write correctness) is drained up to
two units -- at most one PE-heavy attn@v chain -- per score-tile slot, which
keeps the ACT-paced exp pipeline fed. Chains are split into vh/vl halves,
the next batch's qkv/v prologue is fed in per-group chunks, and the final
group's chains run pair-by-pair inside its own t loop so the tail after the
last exp is only the B-head finishes, the norm, and the projection finals.
The cost-model timeline is ~124us/core vs ~100us of irreducible ACT-exp
time (the hard floor: 6.3M exps/batch at 1 elem/cycle/partition).
"""
import sys

sys.path.insert(0, "/opt/trn_rl_repo")
import numpy as np
import concourse.bass as bass
import concourse.tile as tile
from concourse import mybir
from concourse.bass import ts
from concourse.vector_clock import ScopedClock

f32 = mybir.dt.float32
f32r = mybir.dt.float32r
E4 = mybir.dt.float8e4
BF = mybir.dt.bfloat16

N, C, HW, S = 16, 384, 32, 1024
NH, HD = 6, 64
N3C = 3 * C  # 1152
N_CORES = 8
BPC = N // N_CORES  # batches per core
SCALE = HD**-0.5
MM_DT = f32r  # matmul dtype: f32r = full-rate tf32-like; f32 = 1/4-rate, precise
DR = mybir.MatmulPerfMode.DoubleRow
VPAD = 80  # per-head v stride in bytes: dual-fp8 LDWEIGHTS needs 16B-aligned steps

# ---------------------------------------------------------------------------
# Workarounds for walrus 1-sync-wait-per-instruction limit
# ---------------------------------------------------------------------------


def _patched_drain_and_barrier(self, tick_clock, wait_clock):
    nc = self.nc
    probe = nc.sync.nop(nofuse=True, hint="drain_waits")
    wait_clock.add_sem_waits(probe.ins, ScopedClock({None: tick_clock.global_clock}))
    inst = probe.ins
    si = inst.sync_info
    waits = list(si.on_wait) if si is not None else []
    if len(waits) > 1:
        inst.sync_info = mybir.SyncInfo(on_wait=[waits[0]], on_update=list(si.on_update))
        for w in waits[1:]:
            extra = nc.sync.nop(nofuse=True, hint="drain_waits")
            extra.ins.sync_info = mybir.SyncInfo(on_wait=[w], on_update=[])
    nc.sync.drain()
    nc.all_engine_barrier()
    assert self.sems is not None
    popped = nc._tile_sem_poison_stack.pop()
    assert popped is self._sem_poison
    nc.clear_and_free_semaphores(list(self.sems.allocated().values()))
    nc.all_engine_barrier()


tile.TileContext._drain_and_barrier = _patched_drain_and_barrier


def _split_multi_waits(nc):
    n_split = 0
    for fn in nc.m.functions:
        for bb in fn.blocks:
            insts = list(bb.instructions)
            out = []
            changed = False
            for inst in insts:
                si = getattr(inst, "sync_info", None)
                try:
                    waits = list(si.on_wait) if si is not None else []
                except Exception:
                    waits = []
                if len(waits) > 1:
                    for w in waits[:-1]:
                        nop = mybir.InstNoOp(name=f"waitsplit-{n_split}")
                        n_split += 1
                        nop.engine = inst.engine
                        nop.sync_info = mybir.SyncInfo(on_wait=[w], on_update=[])
                        out.append(nop)
                    inst.sync_info = mybir.SyncInfo(
                        on_wait=[waits[-1]], on_update=list(si.on_update)
                    )
                    changed = True
                out.append(inst)
            if changed:
                bb.instructions = out
    return n_split


# ---------------------------------------------------------------------------
# Kernel build
# ---------------------------------------------------------------------------


def _build(iters=1):
    nc = bass.Bass("TRN2", target_bir_lowering=False, debug=False, num_devices=N_CORES)
    xs = nc.declare_dram_parameter("xs", [BPC, C, S], BF, isOutput=False)
    wqkvT_d = nc.declare_dram_parameter("wqkvT", [C, N3C], BF, isOutput=False)
    woutT_d = nc.declare_dram_parameter("woutT", [C, C], MM_DT, isOutput=False)
    bout_d = nc.declare_dram_parameter("bout", [C], f32, isOutput=False)
    esel_d = nc.declare_dram_parameter("esel", [65, 256], MM_DT, isOutput=False)
    out_d = nc.declare_dram_parameter("out", [BPC, C, S], f32, isOutput=True)

    with tile.TileContext(nc) as tc:
        with nc.allow_low_precision(reason="f32r/fp8 matmul pipeline"):
            _emit(nc, tc, xs, wqkvT_d, woutT_d, bout_d, esel_d, out_d, iters)
    _split_multi_waits(nc)
    return nc


def _emit(nc, tc, xs, wqkvT_d, woutT_d, bout_d, esel_d, out_d, iters=1):
    import collections
    import contextlib

    ctx = contextlib.ExitStack()
    consts = ctx.enter_context(tc.tile_pool(name="consts", bufs=1))
    xpool = ctx.enter_context(tc.tile_pool(name="xpool", bufs=2))
    qkpool = ctx.enter_context(tc.tile_pool(name="qkpool", bufs=2))
    vpool = ctx.enter_context(tc.tile_pool(name="vpool", bufs=2))
    apool = ctx.enter_context(tc.tile_pool(name="apool", bufs=4))
    catpool = ctx.enter_context(tc.tile_pool(name="catpool", bufs=2))
    rtpool = ctx.enter_context(tc.tile_pool(name="rtpool", bufs=4))
    fpool = ctx.enter_context(tc.tile_pool(name="fpool", bufs=4))
    ps_wide = ctx.enter_context(tc.tile_pool(name="ps_wide", bufs=2, space="PSUM"))
    ps_o = ctx.enter_context(tc.tile_pool(name="ps_o", bufs=2, space="PSUM"))
    ps_small = ctx.enter_context(tc.tile_pool(name="ps_small", bufs=2, space="PSUM"))


    # ---- constants ----
    # urgent halves (columns 0:512 = q + first k half-block) go out first on
    # the three HWDGE queues; the rest (k tail + v columns) loads as filler
    wq = consts.tile([128, 3, N3C], BF)  # w_qkv^T, bf16: halves DMA bytes, same PE rate
    qengs = (nc.sync, nc.scalar, nc.gpsimd)
    wo = consts.tile([128, 3, C], MM_DT)  # w_out^T
    bo = consts.tile([128, 3], f32)
    # e1[0, 0:64]=1 / e1[0, 128+64:]=1: K=1 selector rows that broadcast the
    # two heads' reciprocal rows (living on partition 64) across catT partitions
    e1 = consts.tile([65, 256], MM_DT)

    # warm the ACT exp table-set during the startup DMA wait (the pseudo
    # table-load walrus inserts before the first Exp costs ~2.7us; without
    # this it lands right when the first scores psum becomes ready)
    actwarm = consts.tile([1, 8], f32)
    nc.vector.memset(actwarm, 0.0)
    nc.scalar.activation(
        out=actwarm, in_=actwarm, func=mybir.ActivationFunctionType.Exp, scale=1.0
    )

    def _load_late_consts():
        for k in range(3):
            qengs[k].dma_start(out=wq[:, k, 512:N3C], in_=wqkvT_d[ts(k, 128), 512:N3C])
        for k in range(3):
            nc.gpsimd.dma_start(out=wo[:, k, :], in_=woutT_d[ts(k, 128), :])
            nc.gpsimd.dma_start(
                out=bo[:, k : k + 1],
                in_=bout_d[ts(k, 128)].rearrange("(p o) -> p o", o=1),
            )
        nc.gpsimd.dma_start(out=e1, in_=esel_d[:, :])

    mm = nc.tensor.matmul
    EXP = mybir.ActivationFunctionType.Exp

    nseq = iters * BPC
    state = {}  # seq -> dict(x, qkT, vh, vl, cat, rt)
    # single FIFO: queue order == emission order == the only safe order
    # (reads must never be emitted before the writes they depend on);
    # entries are (is_heavy_chain, closure)
    filler = collections.deque()
    pending_pro = collections.Counter()  # seq -> un-run prologue closures

    def push(fn, chain=False):
        filler.append((chain, fn))

    def drain(k=1):
        for _ in range(k):
            if filler:
                filler.popleft()[1]()

    def drain_slot():
        # one t-slot of filler work: up to two units, but at most one
        # (PE-heavy) attn@v chain, so group-boundary chain bursts don't
        # starve the ACT-paced scores pipeline; never skips ahead in the
        # queue -- order is correctness
        did_chain = False
        for _ in range(2):
            if not filler:
                return
            is_chain, fn = filler[0]
            if is_chain and did_chain:
                return
            filler.popleft()
            fn()
            did_chain = did_chain or is_chain

    def force_prologue(seq, leave):
        while pending_pro[seq] > leave:
            filler.popleft()[1]()

    def queue_prologue(seq):
        b = seq % BPC
        st = state.setdefault(seq, {})

        units = []

        def xload():
            x_sb = xpool.tile([128, 3, S], BF, tag="x", name=f"x_{seq}")
            # with bf16 the transfers are cheaper than their descriptor
            # generations, so one full-width DMA per k beats split halves
            if seq == 0:
                for k in range(3):
                    qengs[k].dma_start(out=wq[:, k, 0:512], in_=wqkvT_d[ts(k, 128), 0:512])
            for k in range(3):
                qengs[k].dma_start(out=x_sb[:, k, :], in_=xs[b, ts(k, 128), :])
            st["x"] = x_sb
            st["qkT"] = qkpool.tile([128, 6, S], MM_DT, tag="qkT", name=f"qkT_{seq}")
            vh = vpool.tile([128, 8, NH, VPAD], E4, tag="vh", name=f"vh_{seq}")
            vl = vpool.tile([128, 8, NH, VPAD], E4, tag="vl", name=f"vl_{seq}")
            # ones column in the hi chain collects the softmax row sums; the
            # lo chain must contribute zero there
            nc.vector.memset(vh[:, :, :, HD : HD + 1], 1.0)
            nc.vector.memset(vl[:, :, :, HD : HD + 1], 0.0)
            st["vh"], st["vl"] = vh, vl
            pending_pro[seq] -= 1

        units.append(xload)
        if seq == 0:
            units.append(_load_late_consts)

        def qk_unit(j, u):
            def f():
                x_sb, qkT = st["x"], st["qkT"]
                pq = ps_small.tile([128, 512], f32, tag="small", name=f"pq_{seq}_{j}_{u}")
                for k in range(3):
                    mm(pq, wq[:, k, ts(j, 128)], x_sb[:, k, ts(u, 512)],
                       start=(k == 0), stop=(k == 2))
                nc.vector.tensor_copy(out=qkT[:, j, ts(u, 512)], in_=pq)
                pending_pro[seq] -= 1

            return f

        # q/k halves for group g are columns g and 3+g; scores t<4 only read
        # the u=0 half of qT, so (j0,u0),(j3,u0),(j0,u1),(j3,u1) unblocks g=0
        for j, u in ((0, 0), (3, 0), (0, 1), (3, 1), (1, 0), (4, 0), (1, 1), (4, 1),
                     (2, 0), (5, 0), (2, 1), (5, 1)):
            units.append(qk_unit(j, u))

        def v_group(i):
            def f():
                x_sb, vh, vl = st["x"], st["vh"], st["vl"]
                pv = ps_small.tile([128, C], f32, tag="small", name=f"pv_{seq}_{i}")
                for k in range(3):
                    mm(pv, x_sb[:, k, ts(i, 128)], wq[:, k, 2 * C : N3C],
                       start=(k == 0), stop=(k == 2))
                pvh = pv.rearrange("p (h d) -> p h d", h=NH)
                nc.vector.tensor_copy(out=vh[:, i, :, 0:HD], in_=pvh)
                nc.vector.tensor_sub(out=vl[:, i, :, 0:HD], in0=pvh, in1=vh[:, i, :, 0:HD])
                pending_pro[seq] -= 1

            return f

        for i in range(8):
            units.append(v_group(i))
        pending_pro[seq] = 21
        return units

    def queue_norm(seq, g):
        st = state[seq]
        rtA, rtB = st["rt"][g]

        def f():
            catT = st["cat"]
            for u in range(2):
                pr = ps_small.tile([128, 512], f32, tag="small", name=f"pr_{seq}_{g}_{u}")
                mm(pr, e1[64:65, 0:128], rtA[64:65, ts(u, 512)],
                   start=True, stop=False, tile_position=(64, 0))
                mm(pr, e1[64:65, 128:256], rtB[64:65, ts(u, 512)],
                   start=False, stop=True, tile_position=(64, 0))
                nc.vector.tensor_mul(
                    out=catT[:, g, ts(u, 512)], in0=catT[:, g, ts(u, 512)], in1=pr
                )

        push(f)

    def queue_proj_partials(seq):
        st = state[seq]
        st["fin"] = {}

        def partial_unit(j, u):
            def f():
                catT = st["cat"]
                if u == 0:
                    st["fin"][j] = fpool.tile(
                        [128, 1024], f32, tag="fin", name=f"fin_{seq}_{j}"
                    )
                fin = st["fin"][j]
                pf = ps_small.tile([128, 512], f32, tag="small", name=f"pp_{seq}_{j}_{u}")
                for k in range(2):
                    mm(pf, wo[:, k, ts(j, 128)], catT[:, k, ts(u, 512)],
                       start=(k == 0), stop=(k == 1))
                nc.vector.tensor_scalar_add(
                    out=fin[:, ts(u, 512)], in0=pf, scalar1=bo[:, j : j + 1]
                )

            return f

        for j in range(3):
            for u in range(2):
                push(partial_unit(j, u))

    def queue_epilogue(seq):
        b = seq % BPC
        st = state[seq]
        last = seq == nseq - 1

        def final_unit(j, u):
            def f():
                catT = st["cat"]
                fin = st["fin"][j]
                pf = ps_small.tile([128, 512], f32, tag="small", name=f"pf_{seq}_{j}_{u}")
                mm(pf, wo[:, 2, ts(j, 128)], catT[:, 2, ts(u, 512)], start=True, stop=True)
                nc.vector.tensor_add(out=fin[:, ts(u, 512)], in0=fin[:, ts(u, 512)], in1=pf)
                eng = (nc.gpsimd, nc.scalar, nc.sync)[j]
                eng.dma_start(out=out_d[b, ts(j, 128), ts(u, 512)], in_=fin[:, ts(u, 512)])
                # free catT/rt state once the last projection is emitted
                if (j, u) == (2, 1):
                    state.pop(seq, None)

            return f

        for j in range(3):
            for u in range(2):
                push(final_unit(j, u))

    def chain_finish(st, g, h, attn_unused, rt, u, pso, use_act=False):
        catT = st["cat"]
        po = (h % 2) * 64
        # the last group's copies go to ACT (idle at the tail) so DVE can
        # spend the tail on the reciprocals instead
        if use_act:
            nc.scalar.copy(out=catT[po : po + HD, h // 2, ts(u, 512)], in_=pso[0:HD, :])
        else:
            nc.vector.tensor_copy(out=catT[po : po + HD, h // 2, ts(u, 512)], in_=pso[0:HD, :])
        # reciprocal of the sums row stays on partition 64; the norm matmul
        # reads it there via a K=1 selector at tile_position row 64
        nc.vector.reciprocal(out=rt[64:65, ts(u, 512)], in_=pso[HD : HD + 1, :])

    def make_chain(seq, g, h, attn, rt, u):
        """Two half-units (vh pass, then vl pass + finish) sharing one psum
        accumulator, so a slot can hold two halves of different chains."""
        st = state[seq]
        use_act = seq == nseq - 1 and g == 2
        box = {}

        def fh():
            vh = st["vh"]
            box["pso"] = pso = ps_o.tile(
                [HD + 1, 512], f32, tag="o", name=f"pso_{seq}_{h}_{u}"
            )
            for p in range(4):
                mm(pso, vh[:, 2 * p : 2 * p + 2, h, 0 : HD + 1],
                   attn[:, 2 * p : 2 * p + 2, ts(u, 512)],
                   start=(p == 0), stop=False, perf_mode=DR)

        def fl():
            vl = st["vl"]
            pso = box["pso"]
            for p in range(4):
                mm(pso, vl[:, 2 * p : 2 * p + 2, h, 0 : HD + 1],
                   attn[:, 2 * p : 2 * p + 2, ts(u, 512)],
                   start=False, stop=(p == 3), perf_mode=DR)
            chain_finish(st, g, h, attn, rt, u, pso, use_act=use_act)

        return fh, fl

    def make_chain_split(seq, g, h, attn, rt, u, pool, tag):
        """Pair-split variant for the final group: pairs 0-1 run inside the
        t loop (ready after exp t=3), pairs 2-3 + finish after it."""
        st = state[seq]
        box = {}

        def f1():
            vh, vl = st["vh"], st["vl"]
            box["pso"] = pso = pool.tile(
                [HD + 1, 512], f32, tag=tag, name=f"psos_{seq}_{h}_{u}"
            )
            for p in range(2):
                mm(pso, vh[:, 2 * p : 2 * p + 2, h, 0 : HD + 1],
                   attn[:, 2 * p : 2 * p + 2, ts(u, 512)],
                   start=(p == 0), stop=False, perf_mode=DR)
                mm(pso, vl[:, 2 * p : 2 * p + 2, h, 0 : HD + 1],
                   attn[:, 2 * p : 2 * p + 2, ts(u, 512)],
                   start=False, stop=False, perf_mode=DR)

        def mkpair(p, fin):
            def f():
                vh, vl = st["vh"], st["vl"]
                pso = box["pso"]
                mm(pso, vh[:, 2 * p : 2 * p + 2, h, 0 : HD + 1],
                   attn[:, 2 * p : 2 * p + 2, ts(u, 512)],
                   start=False, stop=False, perf_mode=DR)
                mm(pso, vl[:, 2 * p : 2 * p + 2, h, 0 : HD + 1],
                   attn[:, 2 * p : 2 * p + 2, ts(u, 512)],
                   start=False, stop=(p == 3), perf_mode=DR)
                if fin:
                    chain_finish(st, g, h, attn, rt, u, pso, use_act=True)

            return f

        return f1, mkpair(2, False), mkpair(3, True)

    def emit_groups(seq, next_units=None):
        st = state[seq]
        qkT = st["qkT"]
        catT = catpool.tile([128, 3, S], MM_DT, tag="cat", name=f"cat_{seq}")
        st["cat"] = catT
        st["rt"] = {}
        vh, vl = st["vh"], st["vl"]

        for g in range(3):
            hA, hB = 2 * g, 2 * g + 1
            attnA = apool.tile([128, 8, S], E4, tag="attn", name=f"aA_{seq}_{g}")
            attnB = apool.tile([128, 8, S], E4, tag="attn", name=f"aB_{seq}_{g}")
            rtA = rtpool.tile([65, S], MM_DT, tag="rt", name=f"rtA_{seq}_{g}")
            rtB = rtpool.tile([65, S], MM_DT, tag="rt", name=f"rtB_{seq}_{g}")
            st["rt"][g] = (rtA, rtB)
            last_group = seq == nseq - 1 and g == 2
            if last_group:
                sA = [make_chain_split(seq, g, hA, attnA, rtA, u, ps_o, "o")
                      for u in range(2)]
                sB = [make_chain_split(seq, g, hB, attnB, rtB, u, ps_small, "small")
                      for u in range(2)]
            for t in range(8):
                wA = ps_wide.tile([128, 1024], f32, tag="wide", name=f"wA_{seq}_{g}_{t}")
                wB = ps_wide.tile([128, 1024], f32, tag="wide", name=f"wB_{seq}_{g}_{t}")
                if seq == 0 and g == 0 and t == 0:
                    # cold start: the u=1 halves of x/qkT land late, so run
                    # four 512-wide exps, starting on the u=0 halves while
                    # the u=1 qk units are still being emitted
                    for u in range(2):
                        mm(wA[:, ts(u, 512)], qkT[0:64, 3 + g, ts(t, 128)],
                           qkT[0:64, g, ts(u, 512)],
                           start=True, stop=True, tile_position=(0, 0))
                        nc.scalar.activation(out=attnA[:, t, ts(u, 512)],
                                             in_=wA[:, ts(u, 512)], func=EXP, scale=SCALE)
                        mm(wB[:, ts(u, 512)], qkT[64:128, 3 + g, ts(t, 128)],
                           qkT[64:128, g, ts(u, 512)],
                           start=True, stop=True, tile_position=(64, 0))
                        nc.scalar.activation(out=attnB[:, t, ts(u, 512)],
                                             in_=wB[:, ts(u, 512)], func=EXP, scale=SCALE)
                    drain_slot()
                    continue
                mm(wA[:, 0:512], qkT[0:64, 3 + g, ts(t, 128)], qkT[0:64, g, 0:512],
                   start=True, stop=True, tile_position=(0, 0))
                mm(wA[:, 512:1024], qkT[0:64, 3 + g, ts(t, 128)], qkT[0:64, g, 512:1024],
                   start=True, stop=True, tile_position=(0, 0))
                mm(wB[:, 0:512], qkT[64:128, 3 + g, ts(t, 128)], qkT[64:128, g, 0:512],
                   start=True, stop=True, tile_position=(64, 0))
                nc.scalar.activation(out=attnA[:, t, :], in_=wA, func=EXP, scale=SCALE)
                mm(wB[:, 512:1024], qkT[64:128, 3 + g, ts(t, 128)], qkT[64:128, g, 512:1024],
                   start=True, stop=True, tile_position=(64, 0))
                nc.scalar.activation(out=attnB[:, t, :], in_=wB, func=EXP, scale=SCALE)
                if last_group and t == 3:
                    for u in range(2):
                        push(sA[u][0])
                if last_group and t == 5:
                    for u in range(2):
                        push(sB[u][0])
                    for u in range(2):
                        push(sA[u][1])
                if last_group and t == 7:
                    for u in range(2):
                        push(sB[u][1])
                    for u in range(2):
                        push(sA[u][2])
                if t == 7:
                    drain(1)
                else:
                    drain_slot()
            if last_group:
                for u in range(2):
                    push(sB[u][2])
            else:
                cA = [make_chain(seq, g, hA, attnA, rtA, u) for u in range(2)]
                cB = [make_chain(seq, g, hB, attnB, rtB, u) for u in range(2)]
                for pair in (cA, cB):
                    push(pair[0][0])
                    push(pair[1][0])
                    push(pair[0][1])
                    push(pair[1][1])
            queue_norm(seq, g)
            if g == 1:
                queue_proj_partials(seq)
            if next_units:
                n = (10, 11, 0)[g] if len(next_units) == 21 or g else len(next_units)
                for fn in next_units[:n]:
                    push(fn)
                del next_units[:n]

    # ---------------- the pipeline ----------------
    for fn in queue_prologue(0):
        push(fn)
    for seq in range(nseq):
        # xload + the two (j in {0,3}, u=0) qk units unblock the first
        # score tile; its u=1 halves drain inside t=0
        force_prologue(seq, 16)
        next_units = queue_prologue(seq + 1) if seq + 1 < nseq else None
        emit_groups(seq, next_units)
        queue_epilogue(seq)
    drain(len(filler))

    ctx.close()


_CACHED = None


def _get_nc():
    global _CACHED
    if _CACHED is None:
        _CACHED = _build()
    return _CACHED


def _esel_np():
    e = np.zeros((65, 256), np.float32)
    e[64, 0:64] = 1.0
    e[64, 192:256] = 1.0
    return e


def _in_maps(x, w_qkv, w_out, b_out):
    import ml_dtypes

    x = np.ascontiguousarray(np.asarray(x, dtype=np.float32))
    xs_full = x.reshape(N, C, S).astype(ml_dtypes.bfloat16)
    wqkvT = np.ascontiguousarray(np.asarray(w_qkv, np.float32).T).astype(ml_dtypes.bfloat16)
    woutT = np.ascontiguousarray(np.asarray(w_out, np.float32).T)
    bout = np.ascontiguousarray(np.asarray(b_out, np.float32))
    esel = _esel_np()
    return [
        {
            "xs": xs_full[i * BPC : (i + 1) * BPC],
            "wqkvT": wqkvT,
            "woutT": woutT,
            "bout": bout,
            "esel": esel,
        }
        for i in range(N_CORES)
    ]


def kernel(x, w_qkv, w_out, b_out):
    from concourse.bass_utils import run_bass_kernel_spmd

    nc = _get_nc()
    res = run_bass_kernel_spmd(nc, _in_maps(x, w_qkv, w_out, b_out), list(range(N_CORES)))
    out = np.concatenate([res.results[i]["out"] for i in range(N_CORES)], axis=0)
    return out.reshape(N, C, HW, HW)
